# revision 1
# baseline (speedup 1.0000x reference)
"""Trainium2 Bass kernel for deformable attention.

Contract: kernel(**inputs) takes the FULL inputs (as produced by the problem's
setup_inputs) and returns the FULL [4, 1024, 256] float32 output. Internally the
work is sharded over 8 NeuronCores: core c handles batch c//2 and query half
c%2 (512 queries), with the batch's full value feature map replicated on the
core.

Per-core pipeline (all shapes hardcoded for B=4, Q=1024, D=256, H=W=128,
nh=8, npts=4):
  1. The value projection W_v commutes past the (linear) bilinear/attention
     reduce, so it is folded into the output projection on the host:
     Wcomb_h = W_v @ W_out_h and bvW_h = b_v @ W_out_h, with a per-(q,h)
     sum-of-weights term correcting the bias at zero-padded borders. The
     kernel therefore gathers raw bf16 value rows -- no feature-map GEMM.
  2. Coefficient chain: offsets/attention GEMMs + softmax + bilinear weight
     computation, in [query-partition, sample-free] layout, fp32. Per q-tile
     of 128 queries; gather indices for a tile ship as soon as it finishes.
  3. Gather: per (query, head, point, row-corner) descriptor, one dma_gather
     element of 512 bf16 values = two adjacent columns at one row of the
     value map (overlapping row-pair access pattern; 1024 idxs per call --
     larger calls crash the hardware).
  4. Weighted reduce on the TensorEngine: the 128 gathered slots of a query
     pair are the contraction dim (gathered tile is the stationary operand);
     the moving operand is a masked block-diagonal [128, 16] weight matrix
     built from bilinear*attention weights. Output lands as [d, (q, h)] in
     PSUM, which is exactly the lhsT layout the final GEMM needs.
  5. out = weighted @ Wcomb + sw * bvW + b_out.
"""

from contextlib import ExitStack

import numpy as np
import ml_dtypes

NH, NPTS = 8, 4
D = 256
HW = 128            # H == W == 128
NROWS = HW * HW     # 16384
QPC = 512           # queries per core
NCORES = 8
NPAIRS = QPC // 2   # 256 query pairs
NCHUNK = 32         # gather chunks (>1024 idxs per dma_gather crashes HW)
PAIRS_PER_CHUNK = NPAIRS // NCHUNK  # 8
IDX_PER_CHUNK = PAIRS_PER_CHUNK * 128  # 1024

_CACHE = {}


def _mask16_np():
    """[128, 16] bf16: mask[qq*64 + h*8 + p*2 + yp, qq*8 + h] = 1."""
    m = np.zeros((128, 16), dtype=np.float32)
    for qq in range(2):
        for h in range(NH):
            for p in range(NPTS):
                for yp in range(2):
                    m[qq * 64 + h * 8 + p * 2 + yp, qq * 8 + h] = 1.0
    return m.astype(ml_dtypes.bfloat16)


def _build_bass():
    import concourse.bass as bass
    import concourse.bacc as bacc
    import concourse.mybir as mybir
    import concourse.tile as tile
    from concourse.masks import make_identity

    f32 = mybir.dt.float32
    bf16 = mybir.dt.bfloat16
    i16 = mybir.dt.int16
    i32 = mybir.dt.int32
    Alu = mybir.AluOpType
    Act = mybir.ActivationFunctionType

    nc = bacc.Bacc("TRN2", target_bir_lowering=False,
                   dynamic_dma_scratch_size=32768)

    # ---- I/O ----
    query = nc.dram_tensor("query", [QPC, D], f32, kind="ExternalInput")
    refp = nc.dram_tensor("reference_points", [QPC, 2], f32, kind="ExternalInput")
    value = nc.dram_tensor("value", [NROWS, D], bf16, kind="ExternalInput")
    W_off = nc.dram_tensor("W_off", [D, 64], f32, kind="ExternalInput")
    b_off = nc.dram_tensor("b_off", [64], f32, kind="ExternalInput")
    W_attn = nc.dram_tensor("W_attn", [D, 32], f32, kind="ExternalInput")
    b_attn = nc.dram_tensor("b_attn", [32], f32, kind="ExternalInput")
    Wcomb = nc.dram_tensor("Wcomb", [NH * D, D], bf16, kind="ExternalInput")
    bvW = nc.dram_tensor("bvW", [NH, D], bf16, kind="ExternalInput")
    b_out = nc.dram_tensor("b_out", [D], f32, kind="ExternalInput")
    out = nc.dram_tensor("out", [QPC, D], f32, kind="ExternalOutput")

    mask_dram = nc.inline_tensor(_mask16_np(), name="mask16")

    with tile.TileContext(nc) as tc, ExitStack() as ctx:
        sb = ctx.enter_context(tc.tile_pool(name="sb", bufs=1))
        ps = ctx.enter_context(tc.tile_pool(name="ps", bufs=1, space="PSUM"))

        # ---- constants / weights in SBUF ----
        ident = sb.tile([128, 128], f32, tag="ident")
        make_identity(nc, ident[:])
        ones1 = sb.tile([1, 128], f32, tag="ones1")
        nc.vector.memset(ones1[:], 1.0)
        mask16 = sb.tile([128, 16], bf16, tag="mask16")
        nc.sync.dma_start(mask16[:], mask_dram[:])

        wcat = sb.tile([128, 2, 96], f32, tag="wcat")
        nc.sync.dma_start(wcat[:, :, 0:64], W_off[:].rearrange("(t p) n -> p t n", p=128))
        nc.sync.dma_start(wcat[:, :, 64:96], W_attn[:].rearrange("(t p) n -> p t n", p=128))
        nc.vector.tensor_scalar_mul(wcat[:, :, 0:64], wcat[:, :, 0:64], 0.1)
        bias_cat = sb.tile([1, 96], f32, tag="bias_cat")
        nc.sync.dma_start(bias_cat[:, 0:64], b_off[None, :])
        nc.sync.dma_start(bias_cat[:, 64:96], b_attn[None, :])
        nc.vector.tensor_scalar_mul(bias_cat[:, 0:64], bias_cat[:, 0:64], 0.1)

        wout_bf = sb.tile([128, 16, 256], bf16, tag="wout")
        nc.scalar.dma_start(wout_bf[:], Wcomb[:].rearrange("(t p) n -> p t n", p=128))
        bvw_bf = sb.tile([8, 256], bf16, tag="bvw")
        nc.scalar.dma_start(bvw_bf[:], bvW[:])
        bout_sb = sb.tile([1, 256], f32, tag="bout")
        nc.sync.dma_start(bout_sb[:], b_out[None, :])

        # persistent intermediates
        wabT = sb.tile([128, 512], f32, tag="wabT")       # [(AB,h,p,yp), q]
        idxT = sb.tile([64, 512], i16, tag="idxT")        # [(h,p,yp), q]
        w_a_i = sb.tile([128, 256], bf16, tag="w_a_i")    # [(qq,s64), pair]
        w_b_i = sb.tile([128, 256], bf16, tag="w_b_i")
        idxt = sb.tile([128, 256, 2, 4], i16, tag="idxt")  # wrapped gather indices
        red = sb.tile([128, 2, 512, 8], bf16, tag="red")  # [dlo, dh, q, h]
        swT = sb.tile([8, 512], bf16, tag="swT")          # sum of weights [h, q]

        # ================= coefficient phase (4 q-tiles of 128) =============
        # pass 1: offsets GEMM + gather-index path per q-tile (ships indices
        # as early as possible); pass 2 below computes the weights.
        qt_state = []
        for qt in range(4):
            qsl = slice(qt * 128, (qt + 1) * 128)
            q_sb = sb.tile([128, 256], f32, tag="q_sb", bufs=4)
            nc.sync.dma_start(q_sb[:], query[qsl, :])
            rp = sb.tile([128, 2], f32, tag="rp", bufs=4)
            nc.sync.dma_start(rp[:], refp[qsl, :])

            pst = ps.tile([128, 256], f32, tag="tp", bufs=2)
            nc.tensor.transpose(pst[:, 0:128], q_sb[:, 0:128], ident[:])
            nc.tensor.transpose(pst[:, 128:256], q_sb[:, 128:256], ident[:])
            qT = sb.tile([128, 2, 128], f32, tag="qT", bufs=4)
            nc.vector.tensor_copy(qT[:], pst[:])

            psc = ps.tile([128, 96], f32, tag="tp", bufs=2)
            nc.tensor.matmul(psc[:], qT[:, 0, :], wcat[:, 0, :], start=True, stop=False)
            nc.tensor.matmul(psc[:], qT[:, 1, :], wcat[:, 1, :], start=False, stop=False)
            nc.tensor.matmul(psc[:], ones1[:], bias_cat[:], start=False, stop=True)
            coef = sb.tile([128, 96], f32, tag="coef_sb", bufs=4)
            nc.vector.tensor_copy(coef[:], psc[:])

            # sampling grid -> bilinear weights, x/y interleaved [128, 32, 2]
            t_u = sb.tile([128, 32, 2], f32, tag="t_u", bufs=4)
            nc.vector.tensor_tensor(
                t_u[:], coef[:, 0:64].rearrange("p (s c) -> p s c", c=2),
                rp[:, None, :].to_broadcast([128, 32, 2]), Alu.add)
            t_c = t_u  # in-place ok per-element
            nc.vector.tensor_scalar(t_c[:], t_u[:], 0.0, 1.0, Alu.max, Alu.min)
            pxs = sb.tile([128, 64], f32, tag="pxs", bufs=4)  # px + 128
            nc.vector.tensor_scalar(pxs[:], t_c[:].rearrange("p a b -> p (a b)"),
                                    128.0, 127.5, Alu.mult, Alu.add)
            ri = sb.tile([128, 64], i32, tag="ri", bufs=4)
            nc.vector.tensor_copy(ri[:], pxs[:])
            rf = sb.tile([128, 64], f32, tag="rf", bufs=4)
            nc.vector.tensor_copy(rf[:], ri[:])
            gt = sb.tile([128, 64], f32, tag="gt", bufs=4)
            nc.vector.tensor_tensor(gt[:], rf[:], pxs[:], Alu.is_gt)
            flr = sb.tile([128, 64], f32, tag="flr", bufs=4)  # floor(px) + 128
            nc.vector.tensor_tensor(flr[:], rf[:], gt[:], Alu.subtract)
            st = sb.tile([128, 64], f32, tag="st", bufs=4)    # clip start + 128
            nc.vector.tensor_scalar(st[:], flr[:], 128.0, 254.0, Alu.max, Alu.min)
            # ---- gather-index path first: this q-tile's gathers can start
            # while the weight path below is still computing ----
            tbase = sb.tile([128, 32], f32, tag="tbase", bufs=4)
            nc.vector.tensor_scalar(
                tbase[:], st[:].rearrange("p (s c) -> p s c", c=2)[:, :, 1],
                128.0, -16512.0, Alu.mult, Alu.add)
            idx64 = sb.tile([128, 32, 2], f32, tag="idx64", bufs=4)
            nc.vector.tensor_tensor(idx64[:, :, 0], tbase[:],
                                    st[:].rearrange("p (s c) -> p s c", c=2)[:, :, 0], Alu.add)
            nc.vector.tensor_scalar_add(idx64[:, :, 1], idx64[:, :, 0], 128.0)
            pst2 = ps.tile([128, 256], f32, tag="tp", bufs=2)
            nc.tensor.transpose(pst2[0:64, 128:256],
                                idx64[:].rearrange("p s c -> p (s c)"), ident[:])
            nc.vector.tensor_copy(idxT[:, qt * 128:(qt + 1) * 128], pst2[0:64, 128:256])
            qcs = slice(qt * 128, (qt + 1) * 128)
            jsl = slice(qt * 64, (qt + 1) * 64)
            for g4 in range(4):
                nc.scalar.dma_start(
                    idxt[0:16, jsl, :, g4],
                    idxT[g4 * 16:(g4 + 1) * 16, qcs].rearrange("p (j q) -> p j q", q=2))
            for r in range(1, 8):
                nc.scalar.dma_start(idxt[16 * r:16 * (r + 1), jsl], idxt[0:16, jsl])
            qt_state.append((coef, pxs, flr, st))

        # ---- weight paths for all q-tiles (can trail into the gather phase;
        # only the reduce matmuls consume the weights) ----
        for qt in range(4):
            coef, pxs, flr, st = qt_state[qt]
            jsl = slice(qt * 64, (qt + 1) * 64)
            # softmax + bilinear weights
            expw = sb.tile([128, 8, 4], f32, tag="expw", bufs=4)
            nc.scalar.activation(expw[:], coef[:, 64:96], Act.Exp)
            den = sb.tile([128, 8], f32, tag="den", bufs=4)
            nc.vector.tensor_reduce(den[:], expw[:], axis=mybir.AxisListType.X, op=Alu.add)
            rden = sb.tile([128, 8], f32, tag="rden", bufs=4)
            nc.vector.reciprocal(rden[:], den[:])
            attn = sb.tile([128, 32], f32, tag="attn", bufs=4)
            nc.vector.tensor_tensor(
                attn[:].rearrange("p (h f) -> p h f", f=4), expw[:],
                rden[:, :, None].to_broadcast([128, 8, 4]), Alu.mult)
            w1 = sb.tile([128, 64], f32, tag="w1", bufs=4)
            nc.vector.tensor_tensor(w1[:], pxs[:], flr[:], Alu.subtract)
            dd = sb.tile([128, 64], f32, tag="dd", bufs=4)
            nc.vector.tensor_tensor(dd[:], flr[:], st[:], Alu.subtract)
            m0 = sb.tile([128, 64], f32, tag="m0", bufs=4)
            nc.vector.tensor_scalar(m0[:], dd[:], 0.0, None, Alu.is_equal)
            mneg = sb.tile([128, 64], f32, tag="mneg", bufs=4)
            nc.vector.tensor_scalar(mneg[:], dd[:], -1.0, None, Alu.is_equal)
            mpos = sb.tile([128, 64], f32, tag="mpos", bufs=4)
            nc.vector.tensor_scalar(mpos[:], dd[:], 1.0, None, Alu.is_equal)
            u0 = sb.tile([128, 64], f32, tag="u0", bufs=4)
            nc.vector.tensor_scalar(u0[:], w1[:], 1.0, -1.0, Alu.subtract, Alu.mult)
            # wA = u0*m0 + u1*mneg ; wB = u1*m0 + u0*mpos   (u1 == w1)
            tA = sb.tile([128, 64], f32, tag="tA", bufs=4)
            nc.vector.tensor_tensor(tA[:], u0[:], m0[:], Alu.mult)
            tB = sb.tile([128, 64], f32, tag="tB", bufs=4)
            nc.vector.tensor_tensor(tB[:], w1[:], mneg[:], Alu.mult)
            wA = sb.tile([128, 32, 2], f32, tag="wA", bufs=4)
            nc.vector.tensor_tensor(wA[:].rearrange("p a b -> p (a b)"), tA[:], tB[:], Alu.add)
            nc.vector.tensor_tensor(tA[:], w1[:], m0[:], Alu.mult)
            nc.vector.tensor_tensor(tB[:], u0[:], mpos[:], Alu.mult)
            wB = sb.tile([128, 32, 2], f32, tag="wB", bufs=4)
            nc.vector.tensor_tensor(wB[:].rearrange("p a b -> p (a b)"), tA[:], tB[:], Alu.add)

            # combine with attention; build wab [128, (AB, h*p, yp)]
            aw = sb.tile([128, 32], f32, tag="aw", bufs=4)
            nc.vector.tensor_tensor(aw[:], attn[:], wA[:, :, 0], Alu.mult)
            bw = sb.tile([128, 32], f32, tag="bw", bufs=4)
            nc.vector.tensor_tensor(bw[:], attn[:], wB[:, :, 0], Alu.mult)
            vcat = sb.tile([128, 32, 2], f32, tag="vcat", bufs=4)
            nc.vector.tensor_copy(vcat[:, :, 0], wA[:, :, 1])
            nc.vector.tensor_copy(vcat[:, :, 1], wB[:, :, 1])
            wab = sb.tile([128, 2, 32, 2], f32, tag="wab", bufs=4)
            nc.vector.tensor_tensor(wab[:, 0], vcat[:],
                                    aw[:, :, None].to_broadcast([128, 32, 2]), Alu.mult)
            nc.vector.tensor_tensor(wab[:, 1], vcat[:],
                                    bw[:, :, None].to_broadcast([128, 32, 2]), Alu.mult)

            # sum of all weights per (q, h) -- border-clip correction for the
            # folded b_v term: sw = sum_{AB,p,yp} wab
            swq = sb.tile([128, 8], f32, tag="swq", bufs=4)
            nc.vector.tensor_reduce(
                swq[:], wab[:].rearrange("p a (h r) c -> p h a r c", h=8),
                axis=mybir.AxisListType.XYZ, op=Alu.add)

            pst2b = pst2
            nc.tensor.transpose(pst2b[:, 0:128],
                                wab[:].rearrange("p a s c -> p (a s c)"), ident[:])
            pst3 = ps.tile([8, 128], f32, tag="img", bufs=2)
            nc.tensor.transpose(pst3[:], swq[:], ident[:])
            nc.vector.tensor_copy(wabT[:, qt * 128:(qt + 1) * 128], pst2b[:, 0:128])
            nc.vector.tensor_copy(swT[:, qt * 128:(qt + 1) * 128], pst3[:])
            nc.vector.tensor_copy(w_a_i[0:64, jsl], wabT[0:64, qt * 128:qt * 128 + 128:2])
            nc.vector.tensor_copy(w_a_i[64:128, jsl], wabT[0:64, qt * 128 + 1:qt * 128 + 128:2])
            nc.vector.tensor_copy(w_b_i[0:64, jsl], wabT[64:128, qt * 128:qt * 128 + 128:2])
            nc.vector.tensor_copy(w_b_i[64:128, jsl], wabT[64:128, qt * 128 + 1:qt * 128 + 128:2])

        # ================= gather + weighted reduce =========================
        import concourse.bass as bass_mod
        gather_src = bass_mod.AP(
            tensor=value, offset=0, ap=[[256, NROWS - 1], [1, 512]])

        for g in range(NCHUNK):
            gsl = slice(g * PAIRS_PER_CHUNK, (g + 1) * PAIRS_PER_CHUNK)
            gt_sb = sb.tile([128, PAIRS_PER_CHUNK, 512], bf16, tag="gat", bufs=12)
            nc.gpsimd.dma_gather(
                out_ap=gt_sb[:],
                in_ap=gather_src,
                idxs_ap=idxt[:].rearrange("p a b c -> p (a b c)")[
                    :, g * (IDX_PER_CHUNK // 16):(g + 1) * (IDX_PER_CHUNK // 16)],
                num_idxs=IDX_PER_CHUNK,
                num_idxs_reg=IDX_PER_CHUNK,
                elem_size=512,
                elem_step=256,
            )
            wblkA = sb.tile([128, PAIRS_PER_CHUNK, 16], bf16, tag="wblkA", bufs=4)
            nc.vector.tensor_tensor(
                wblkA[:], mask16[:, None, :].to_broadcast([128, PAIRS_PER_CHUNK, 16]),
                w_a_i[:, gsl, None].to_broadcast([128, PAIRS_PER_CHUNK, 16]), Alu.mult)
            wblkB = sb.tile([128, PAIRS_PER_CHUNK, 16], bf16, tag="wblkB", bufs=4)
            nc.vector.tensor_tensor(
                wblkB[:], mask16[:, None, :].to_broadcast([128, PAIRS_PER_CHUNK, 16]),
                w_b_i[:, gsl, None].to_broadcast([128, PAIRS_PER_CHUNK, 16]), Alu.mult)

            plo = ps.tile([128, PAIRS_PER_CHUNK * 16], f32, tag="red_lo", bufs=2)
            phi = ps.tile([128, PAIRS_PER_CHUNK * 16], f32, tag="red_hi", bufs=2)
            for j in range(PAIRS_PER_CHUNK):
                osl = slice(j * 16, (j + 1) * 16)
                nc.tensor.matmul(plo[:, osl], gt_sb[:, j, 0:128], wblkA[:, j, :],
                                 start=True, stop=False)
                nc.tensor.matmul(phi[:, osl], gt_sb[:, j, 128:256], wblkA[:, j, :],
                                 start=True, stop=False)
                nc.tensor.matmul(plo[:, osl], gt_sb[:, j, 256:384], wblkB[:, j, :],
                                 start=False, stop=True)
                nc.tensor.matmul(phi[:, osl], gt_sb[:, j, 384:512], wblkB[:, j, :],
                                 start=False, stop=True)
            qsl2 = slice(g * 2 * PAIRS_PER_CHUNK, (g + 1) * 2 * PAIRS_PER_CHUNK)
            nc.vector.tensor_copy(
                red[:, 0, qsl2, :].rearrange("p a b -> p (a b)"), plo[:])
            nc.vector.tensor_copy(
                red[:, 1, qsl2, :].rearrange("p a b -> p (a b)"), phi[:])

        # ================= out = weighted @ W_out + b_out ===================
        for qt in range(4):
            pso = ps.tile([128, 256], f32, tag="img", bufs=2)
            for kt in range(16):
                h, dh = kt // 2, kt % 2
                lhsT = red[:, dh, qt * 128:(qt + 1) * 128, h]
                nc.tensor.matmul(pso[:], lhsT, wout_bf[:, kt, :],
                                 start=(kt == 0), stop=False)
            nc.tensor.matmul(pso[:], swT[:, qt * 128:(qt + 1) * 128], bvw_bf[:],
                             start=False, stop=False)
            nc.tensor.matmul(pso[:], ones1[:], bout_sb[:], start=False, stop=True)
            o_sb = sb.tile([128, 256], f32, tag="o_sb", bufs=2)
            nc.vector.tensor_copy(o_sb[:], pso[:])
            nc.sync.dma_start(out[qt * 128:(qt + 1) * 128, :], o_sb[:])

    nc.compile()
    return nc


def _get_nc():
    if "nc" not in _CACHE:
        _CACHE["nc"] = _build_bass()
    return _CACHE["nc"]


def _make_in_maps(inputs):
    query = np.ascontiguousarray(np.asarray(inputs["query"], dtype=np.float32))
    refp = np.ascontiguousarray(np.asarray(inputs["reference_points"], dtype=np.float32))
    value = np.ascontiguousarray(
        np.asarray(inputs["value"], dtype=np.float32).astype(ml_dtypes.bfloat16))
    consts = {
        k: np.ascontiguousarray(np.asarray(inputs[k], np.float32))
        for k in ["W_off", "b_off", "W_attn", "b_attn", "b_out"]
    }
    W_v = np.asarray(inputs["W_v"], np.float64)
    b_v = np.asarray(inputs["b_v"], np.float64)
    W_out = np.asarray(inputs["W_out"], np.float64).reshape(NH, D, D)
    consts["Wcomb"] = np.ascontiguousarray(
        np.einsum("ij,hjk->hik", W_v, W_out).reshape(NH * D, D).astype(ml_dtypes.bfloat16))
    consts["bvW"] = np.ascontiguousarray(
        np.einsum("j,hjk->hk", b_v, W_out).astype(ml_dtypes.bfloat16))
    in_maps = []
    for c in range(NCORES):
        b, s = c // 2, c % 2
        qsl = slice(s * QPC, (s + 1) * QPC)
        in_maps.append({
            "query": np.ascontiguousarray(query[b, qsl]),
            "reference_points": np.ascontiguousarray(refp[b, qsl]),
            "value": np.ascontiguousarray(value[b]),
            **consts,
        })
    return in_maps


def _assemble(outs, shape):
    out = np.zeros(shape, dtype=np.float32)
    for c in range(NCORES):
        b, s = c // 2, c % 2
        out[b, s * QPC:(s + 1) * QPC] = outs[c]["out"]
    return out


def kernel(query, reference_points, value, W_off, b_off, W_attn, b_attn,
           W_v, b_v, W_out, b_out, H=128, W=128, **_unused):
    assert int(H) == HW and int(W) == HW
    from concourse.bass_utils import run_bass_kernel_spmd

    inputs = dict(query=query, reference_points=reference_points, value=value,
                  W_off=W_off, b_off=b_off, W_attn=W_attn, b_attn=b_attn,
                  W_v=W_v, b_v=b_v, W_out=W_out, b_out=b_out)
    in_maps = _make_in_maps(inputs)
    nc = _get_nc()
    res = run_bass_kernel_spmd(nc, in_maps, core_ids=list(range(NCORES)))
    outs = res.results if hasattr(res, "results") else res
    B, Q, _ = np.asarray(query).shape
    return _assemble(outs, (B, Q, D))



# revision 13
# speedup vs baseline: 1.5183x; 1.5183x over previous
"""Trainium2 Bass kernel for deformable attention.

Contract: kernel(**inputs) takes the FULL inputs (as produced by the problem's
setup_inputs) and returns the FULL [4, 1024, 256] float32 output. Internally the
work is sharded over 8 NeuronCores: core c handles batch c//2 and query half
c%2 (512 queries), with the batch's full value feature map replicated on the
core.

Per-core pipeline (all shapes hardcoded for B=4, Q=1024, D=256, H=W=128,
nh=8, npts=4):
  1. The value projection W_v commutes past the (linear) bilinear/attention
     reduce, so it is folded into the output projection on the host:
     Wcomb_h = W_v @ W_out_h and bvW_h = b_v @ W_out_h, with a per-(q,h)
     sum-of-weights term correcting the bias at zero-padded borders. The
     kernel therefore gathers raw bf16 value rows -- no feature-map GEMM.
  2. Coefficient chain: offsets/attention GEMMs + softmax + bilinear weight
     computation, in [query-partition, sample-free] layout, fp32. Per q-tile
     of 128 queries; gather indices for a tile ship as soon as it finishes.
  3. Gather: per (query, head, point, row-corner) descriptor, one dma_gather
     element of 512 bf16 values = two adjacent columns at one row of the
     value map (overlapping row-pair access pattern; 1024 idxs per call --
     larger calls crash the hardware).
  4. Weighted reduce on the TensorEngine: the 128 gathered slots of a query
     pair are the contraction dim (gathered tile is the stationary operand);
     the moving operand is a masked block-diagonal [128, 16] weight matrix
     built from bilinear*attention weights. Output lands as [d, (q, h)] in
     PSUM, which is exactly the lhsT layout the final GEMM needs.
  5. out = weighted @ Wcomb + sw * bvW + b_out.
"""

from contextlib import ExitStack

import numpy as np
import ml_dtypes

NH, NPTS = 8, 4
D = 256
HW = 128            # H == W == 128
NROWS = HW * HW     # 16384
QPC = 512           # queries per core
NCORES = 8
NPAIRS = QPC // 2   # 256 query pairs
NCHUNK = 32         # gather chunks (>1024 idxs per dma_gather crashes HW)
PAIRS_PER_CHUNK = NPAIRS // NCHUNK  # 8
IDX_PER_CHUNK = PAIRS_PER_CHUNK * 128  # 1024
VSCALE = 4.0        # value pre-scale before fp8 e3m4 quantization

_CACHE = {}


def _mask16_np():
    """[128, 16] bf16: mask[qq*64 + h*8 + p*2 + yp, qq*8 + h] = 1."""
    m = np.zeros((128, 16), dtype=np.float32)
    for qq in range(2):
        for h in range(NH):
            for p in range(NPTS):
                for yp in range(2):
                    m[qq * 64 + h * 8 + p * 2 + yp, qq * 8 + h] = 1.0
    return m.astype(ml_dtypes.bfloat16)


def _build_bass():
    import concourse.bass as bass
    import concourse.bacc as bacc
    import concourse.mybir as mybir
    import concourse.tile as tile
    from concourse.masks import make_identity

    f32 = mybir.dt.float32
    bf16 = mybir.dt.bfloat16
    f8 = mybir.dt.float8e3
    i16 = mybir.dt.int16
    i32 = mybir.dt.int32
    Alu = mybir.AluOpType
    Act = mybir.ActivationFunctionType

    nc = bacc.Bacc("TRN2", target_bir_lowering=False,
                   dynamic_dma_scratch_size=32768)

    # ---- I/O ----
    query = nc.dram_tensor("query", [QPC, D], f32, kind="ExternalInput")
    refp = nc.dram_tensor("reference_points", [QPC, 2], f32, kind="ExternalInput")
    value = nc.dram_tensor("value", [NROWS, D], f8, kind="ExternalInput")
    W_off = nc.dram_tensor("W_off", [D, 64], f32, kind="ExternalInput")
    b_off = nc.dram_tensor("b_off", [64], f32, kind="ExternalInput")
    W_attn = nc.dram_tensor("W_attn", [D, 32], f32, kind="ExternalInput")
    b_attn = nc.dram_tensor("b_attn", [32], f32, kind="ExternalInput")
    Wcomb = nc.dram_tensor("Wcomb", [NH * D, D], bf16, kind="ExternalInput")
    bvW = nc.dram_tensor("bvW", [NH, D], bf16, kind="ExternalInput")
    b_out = nc.dram_tensor("b_out", [D], f32, kind="ExternalInput")
    out = nc.dram_tensor("out", [QPC, D], f32, kind="ExternalOutput")

    mask_dram = nc.inline_tensor(_mask16_np(), name="mask16")

    with tile.TileContext(nc) as tc, ExitStack() as ctx:
        sb = ctx.enter_context(tc.tile_pool(name="sb", bufs=1))
        ps = ctx.enter_context(tc.tile_pool(name="ps", bufs=1, space="PSUM"))

        # ---- constants / weights in SBUF ----
        ident = sb.tile([128, 128], f32, tag="ident")
        make_identity(nc, ident[:])
        ones1 = sb.tile([1, 128], f32, tag="ones1")
        nc.vector.memset(ones1[:], 1.0)
        mask16 = sb.tile([128, 16], bf16, tag="mask16")
        nc.sync.dma_start(mask16[:], mask_dram[:])

        wcat = sb.tile([128, 2, 96], f32, tag="wcat")
        nc.sync.dma_start(wcat[:, :, 0:64], W_off[:].rearrange("(t p) n -> p t n", p=128))
        nc.sync.dma_start(wcat[:, :, 64:96], W_attn[:].rearrange("(t p) n -> p t n", p=128))
        nc.vector.tensor_scalar_mul(wcat[:, :, 0:64], wcat[:, :, 0:64], 0.1)
        bias_cat = sb.tile([1, 96], f32, tag="bias_cat")
        nc.sync.dma_start(bias_cat[:, 0:64], b_off[None, :])
        nc.sync.dma_start(bias_cat[:, 64:96], b_attn[None, :])
        nc.vector.tensor_scalar_mul(bias_cat[:, 0:64], bias_cat[:, 0:64], 0.1)

        wout_bf = sb.tile([128, 16, 256], bf16, tag="wout")
        nc.scalar.dma_start(wout_bf[:], Wcomb[:].rearrange("(t p) n -> p t n", p=128))
        bvw_bf = sb.tile([8, 256], bf16, tag="bvw")
        nc.scalar.dma_start(bvw_bf[:], bvW[:])
        bout_sb = sb.tile([1, 256], f32, tag="bout")
        nc.sync.dma_start(bout_sb[:], b_out[None, :])

        # persistent intermediates
        wabT = sb.tile([128, 512], f32, tag="wabT")       # [(AB,h,p,yp), q]
        idxT = sb.tile([64, 512], i16, tag="idxT")        # [(h,p,yp), q]
        w_a_i = sb.tile([128, 256], bf16, tag="w_a_i")    # [(qq,s64), pair]
        w_b_i = sb.tile([128, 256], bf16, tag="w_b_i")
        # per-q-tile gather-index tiles: chunk g waits only on tile g//8's
        # writes, so the first gathers fire as soon as q-tile 0's indices land
        idxt_t = []
        for _qt in range(4):
            idxt_q = sb.tile([128, 64, 2, 4], i16, tag="idxt", bufs=4)
            idxt_t.append(idxt_q)
        red = sb.tile([128, 2, 512, 8], bf16, tag="red")  # [dlo, dh, q, h]
        swT = sb.tile([8, 512], bf16, tag="swT")          # sum of weights [h, q]

        # input loads up front (no deps; SP queue stays clear later)
        q_sbs, rps = [], []
        for qt in range(4):
            qsl = slice(qt * 128, (qt + 1) * 128)
            q_sb = sb.tile([128, 256], f32, tag="q_sb", bufs=4)
            nc.sync.dma_start(q_sb[:], query[qsl, :])
            rp = sb.tile([128, 2], f32, tag="rp", bufs=4)
            nc.sync.dma_start(rp[:], refp[qsl, :])
            q_sbs.append(q_sb)
            rps.append(rp)

        # ================= coefficient phase (4 q-tiles of 128) =============
        # pass 1: offsets GEMM + gather-index path per q-tile (ships indices
        # as early as possible); pass 2 below computes the weights.
        qt_state = []
        for qt in range(4):
            q_sb, rp = q_sbs[qt], rps[qt]

            pst = ps.tile([128, 256], f32, tag="tp", bufs=2)
            nc.tensor.transpose(pst[:, 0:128], q_sb[:, 0:128], ident[:])
            nc.tensor.transpose(pst[:, 128:256], q_sb[:, 128:256], ident[:])
            qT = sb.tile([128, 2, 128], f32, tag="qT", bufs=4)
            nc.scalar.copy(qT[:], pst[:])

            psc = ps.tile([128, 96], f32, tag="tp", bufs=2)
            nc.tensor.matmul(psc[:], qT[:, 0, :], wcat[:, 0, :], start=True, stop=False)
            nc.tensor.matmul(psc[:], qT[:, 1, :], wcat[:, 1, :], start=False, stop=False)
            nc.tensor.matmul(psc[:], ones1[:], bias_cat[:], start=False, stop=True)
            coef = sb.tile([128, 96], f32, tag="coef_sb", bufs=4)
            nc.scalar.copy(coef[:], psc[:])

            # sampling grid -> bilinear weights, x/y interleaved [128, 32, 2]
            t_u = sb.tile([128, 32, 2], f32, tag="t_u", bufs=4)
            nc.vector.tensor_tensor(
                t_u[:], coef[:, 0:64].rearrange("p (s c) -> p s c", c=2),
                rp[:, None, :].to_broadcast([128, 32, 2]), Alu.add)
            t_c = t_u  # in-place ok per-element
            nc.vector.tensor_scalar(t_c[:], t_u[:], 0.0, 1.0, Alu.max, Alu.min)
            pxs = sb.tile([128, 64], f32, tag="pxs", bufs=4)  # px + 128
            nc.vector.tensor_scalar(pxs[:], t_c[:].rearrange("p a b -> p (a b)"),
                                    128.0, 127.5, Alu.mult, Alu.add)
            ri = sb.tile([128, 64], i32, tag="ri", bufs=4)
            nc.vector.tensor_copy(ri[:], pxs[:])
            rf = sb.tile([128, 64], f32, tag="rf", bufs=4)
            nc.vector.tensor_copy(rf[:], ri[:])
            gt = sb.tile([128, 64], f32, tag="gt", bufs=4)
            nc.vector.tensor_tensor(gt[:], rf[:], pxs[:], Alu.is_gt)
            flr = sb.tile([128, 64], f32, tag="flr", bufs=4)  # floor(px) + 128
            nc.vector.tensor_tensor(flr[:], rf[:], gt[:], Alu.subtract)
            st = sb.tile([128, 64], f32, tag="st", bufs=4)    # clip start + 128
            nc.vector.tensor_scalar(st[:], flr[:], 128.0, 254.0, Alu.max, Alu.min)
            # ---- gather-index path first: this q-tile's gathers can start
            # while the weight path below is still computing ----
            tbase = sb.tile([128, 32], f32, tag="tbase", bufs=4)
            nc.vector.tensor_scalar(
                tbase[:], st[:].rearrange("p (s c) -> p s c", c=2)[:, :, 1],
                128.0, -16512.0, Alu.mult, Alu.add)
            idx64 = sb.tile([128, 32, 2], f32, tag="idx64", bufs=4)
            nc.vector.tensor_tensor(idx64[:, :, 0], tbase[:],
                                    st[:].rearrange("p (s c) -> p s c", c=2)[:, :, 0], Alu.add)
            nc.vector.tensor_scalar_add(idx64[:, :, 1], idx64[:, :, 0], 128.0)
            pst2 = ps.tile([128, 256], f32, tag="tp", bufs=2)
            nc.tensor.transpose(pst2[0:64, 128:256],
                                idx64[:].rearrange("p s c -> p (s c)"), ident[:])
            nc.vector.tensor_copy(idxT[:, qt * 128:(qt + 1) * 128], pst2[0:64, 128:256])
            qcs = slice(qt * 128, (qt + 1) * 128)
            idxt = idxt_t[qt]
            for g4 in range(4):
                nc.sync.dma_start(
                    idxt[0:16, :, :, g4],
                    idxT[g4 * 16:(g4 + 1) * 16, qcs].rearrange("p (j q) -> p j q", q=2))
            for lo, hi in ((16, 32), (32, 64), (64, 128)):
                nc.sync.dma_start(idxt[lo:hi, :], idxt[0:hi - lo, :])
            qt_state.append((coef, pxs, flr, st))

        # ---- weight paths for all q-tiles (can trail into the gather phase;
        # only the reduce matmuls consume the weights) ----
        for qt in range(4):
            coef, pxs, flr, st = qt_state[qt]
            jsl = slice(qt * 64, (qt + 1) * 64)
            # softmax + bilinear weights
            expw = sb.tile([128, 8, 4], f32, tag="expw", bufs=4)
            nc.scalar.activation(expw[:], coef[:, 64:96], Act.Exp)
            den = sb.tile([128, 8], f32, tag="den", bufs=4)
            nc.vector.tensor_reduce(den[:], expw[:], axis=mybir.AxisListType.X, op=Alu.add)
            rden = sb.tile([128, 8], f32, tag="rden", bufs=4)
            nc.vector.reciprocal(rden[:], den[:])
            attn = sb.tile([128, 32], f32, tag="attn", bufs=4)
            nc.vector.tensor_tensor(
                attn[:].rearrange("p (h f) -> p h f", f=4), expw[:],
                rden[:, :, None].to_broadcast([128, 8, 4]), Alu.mult)
            w1 = sb.tile([128, 64], f32, tag="w1", bufs=4)
            nc.vector.tensor_tensor(w1[:], pxs[:], flr[:], Alu.subtract)
            dd = sb.tile([128, 64], f32, tag="dd", bufs=4)
            nc.vector.tensor_tensor(dd[:], flr[:], st[:], Alu.subtract)
            m0 = sb.tile([128, 64], f32, tag="m0", bufs=4)
            nc.vector.tensor_scalar(m0[:], dd[:], 0.0, None, Alu.is_equal)
            mneg = sb.tile([128, 64], f32, tag="mneg", bufs=4)
            nc.vector.tensor_scalar(mneg[:], dd[:], -1.0, None, Alu.is_equal)
            mpos = sb.tile([128, 64], f32, tag="mpos", bufs=4)
            nc.vector.tensor_scalar(mpos[:], dd[:], 1.0, None, Alu.is_equal)
            u0 = sb.tile([128, 64], f32, tag="u0", bufs=4)
            nc.vector.tensor_scalar(u0[:], w1[:], 1.0, -1.0, Alu.subtract, Alu.mult)
            # wA = u0*m0 + u1*mneg ; wB = u1*m0 + u0*mpos   (u1 == w1)
            tA = sb.tile([128, 64], f32, tag="tA", bufs=4)
            nc.vector.tensor_tensor(tA[:], u0[:], m0[:], Alu.mult)
            tB = sb.tile([128, 64], f32, tag="tB", bufs=4)
            nc.vector.tensor_tensor(tB[:], w1[:], mneg[:], Alu.mult)
            wA = sb.tile([128, 32, 2], f32, tag="wA", bufs=4)
            nc.vector.tensor_tensor(wA[:].rearrange("p a b -> p (a b)"), tA[:], tB[:], Alu.add)
            nc.vector.tensor_tensor(tA[:], w1[:], m0[:], Alu.mult)
            nc.vector.tensor_tensor(tB[:], u0[:], mpos[:], Alu.mult)
            wB = sb.tile([128, 32, 2], f32, tag="wB", bufs=4)
            nc.vector.tensor_tensor(wB[:].rearrange("p a b -> p (a b)"), tA[:], tB[:], Alu.add)

            # combine with attention; build wab [128, (AB, h*p, yp)]
            aw = sb.tile([128, 32], f32, tag="aw", bufs=4)
            nc.vector.tensor_tensor(aw[:], attn[:], wA[:, :, 0], Alu.mult)
            bw = sb.tile([128, 32], f32, tag="bw", bufs=4)
            nc.vector.tensor_tensor(bw[:], attn[:], wB[:, :, 0], Alu.mult)
            vcat = sb.tile([128, 32, 2], f32, tag="vcat", bufs=4)
            nc.vector.tensor_copy(vcat[:, :, 0], wA[:, :, 1])
            nc.vector.tensor_copy(vcat[:, :, 1], wB[:, :, 1])
            wab = sb.tile([128, 2, 32, 2], f32, tag="wab", bufs=4)
            nc.vector.tensor_tensor(wab[:, 0], vcat[:],
                                    aw[:, :, None].to_broadcast([128, 32, 2]), Alu.mult)
            nc.vector.tensor_tensor(wab[:, 1], vcat[:],
                                    bw[:, :, None].to_broadcast([128, 32, 2]), Alu.mult)

            # sum of all weights per (q, h) -- border-clip correction for the
            # folded b_v term: sw = sum_{AB,p,yp} wab
            swq = sb.tile([128, 8], f32, tag="swq", bufs=4)
            nc.vector.tensor_reduce(
                swq[:], wab[:].rearrange("p a (h r) c -> p h a r c", h=8),
                axis=mybir.AxisListType.XYZ, op=Alu.add)

            pst2b = pst2
            nc.tensor.transpose(pst2b[:, 0:128],
                                wab[:].rearrange("p a s c -> p (a s c)"), ident[:])
            pst3 = ps.tile([8, 128], f32, tag="img", bufs=2)
            nc.tensor.transpose(pst3[:], swq[:], ident[:])
            nc.vector.tensor_copy(wabT[:, qt * 128:(qt + 1) * 128], pst2b[:, 0:128])
            nc.vector.tensor_copy(swT[:, qt * 128:(qt + 1) * 128], pst3[:])
            nc.vector.tensor_copy(w_a_i[0:64, jsl], wabT[0:64, qt * 128:qt * 128 + 128:2])
            nc.vector.tensor_copy(w_a_i[64:128, jsl], wabT[0:64, qt * 128 + 1:qt * 128 + 128:2])
            nc.vector.tensor_copy(w_b_i[0:64, jsl], wabT[64:128, qt * 128:qt * 128 + 128:2])
            nc.vector.tensor_copy(w_b_i[64:128, jsl], wabT[64:128, qt * 128 + 1:qt * 128 + 128:2])

        # ================= gather + weighted reduce =========================
        import concourse.bass as bass_mod
        gather_src = bass_mod.AP(
            tensor=value, offset=0, ap=[[256, NROWS - 1], [1, 512]])

        for g in range(NCHUNK):
            gl = g % 8
            gsl = slice(g * PAIRS_PER_CHUNK, (g + 1) * PAIRS_PER_CHUNK)
            gt_sb = sb.tile([128, PAIRS_PER_CHUNK, 512], f8, tag="gat", bufs=12)
            nc.gpsimd.dma_gather(
                out_ap=gt_sb[:],
                in_ap=gather_src,
                idxs_ap=idxt_t[g // 8][:].rearrange("p a b c -> p (a b c)")[
                    :, gl * (IDX_PER_CHUNK // 16):(gl + 1) * (IDX_PER_CHUNK // 16)],
                num_idxs=IDX_PER_CHUNK,
                num_idxs_reg=IDX_PER_CHUNK,
                elem_size=512,
                elem_step=256,
            )
            wblkA = sb.tile([128, PAIRS_PER_CHUNK, 16], bf16, tag="wblkA", bufs=4)
            nc.vector.tensor_tensor(
                wblkA[:], mask16[:, None, :].to_broadcast([128, PAIRS_PER_CHUNK, 16]),
                w_a_i[:, gsl, None].to_broadcast([128, PAIRS_PER_CHUNK, 16]), Alu.mult)
            wblkB = sb.tile([128, PAIRS_PER_CHUNK, 16], bf16, tag="wblkB", bufs=4)
            nc.vector.tensor_tensor(
                wblkB[:], mask16[:, None, :].to_broadcast([128, PAIRS_PER_CHUNK, 16]),
                w_b_i[:, gsl, None].to_broadcast([128, PAIRS_PER_CHUNK, 16]), Alu.mult)

            plo = ps.tile([128, PAIRS_PER_CHUNK * 16], f32, tag="red_lo", bufs=2)
            phi = ps.tile([128, PAIRS_PER_CHUNK * 16], f32, tag="red_hi", bufs=2)
            for j in range(PAIRS_PER_CHUNK):
                osl = slice(j * 16, (j + 1) * 16)
                nc.tensor.matmul(plo[:, osl], gt_sb[:, j, 0:128], wblkA[:, j, :],
                                 start=True, stop=False)
                nc.tensor.matmul(phi[:, osl], gt_sb[:, j, 128:256], wblkA[:, j, :],
                                 start=True, stop=False)
                nc.tensor.matmul(plo[:, osl], gt_sb[:, j, 256:384], wblkB[:, j, :],
                                 start=False, stop=True)
                nc.tensor.matmul(phi[:, osl], gt_sb[:, j, 384:512], wblkB[:, j, :],
                                 start=False, stop=True)
            qsl2 = slice(g * 2 * PAIRS_PER_CHUNK, (g + 1) * 2 * PAIRS_PER_CHUNK)
            nc.scalar.copy(
                red[:, 0, qsl2, :].rearrange("p a b -> p (a b)"), plo[:])
            nc.scalar.copy(
                red[:, 1, qsl2, :].rearrange("p a b -> p (a b)"), phi[:])

            # ---- out = weighted @ W_out + b_out, per q-tile as soon as its
            # 8 chunks are reduced (keeps the PE/out path off the tail) ----
            if gl == 7:
                qt = g // 8
                pso = ps.tile([128, 256], f32, tag="img", bufs=2)
                for kt in range(16):
                    h, dh = kt // 2, kt % 2
                    lhsT = red[:, dh, qt * 128:(qt + 1) * 128, h]
                    nc.tensor.matmul(pso[:], lhsT, wout_bf[:, kt, :],
                                     start=(kt == 0), stop=False)
                nc.tensor.matmul(pso[:], swT[:, qt * 128:(qt + 1) * 128],
                                 bvw_bf[:], start=False, stop=False)
                nc.tensor.matmul(pso[:], ones1[:], bout_sb[:],
                                 start=False, stop=True)
                o_sb = sb.tile([128, 256], f32, tag="o_sb", bufs=2)
                nc.scalar.copy(o_sb[:], pso[:])
                nc.sync.dma_start(out[qt * 128:(qt + 1) * 128, :], o_sb[:])

    nc.compile()
    return nc


def _get_nc():
    if "nc" not in _CACHE:
        _CACHE["nc"] = _build_bass()
    return _CACHE["nc"]


def _make_in_maps(inputs):
    query = np.ascontiguousarray(np.asarray(inputs["query"], dtype=np.float32))
    refp = np.ascontiguousarray(np.asarray(inputs["reference_points"], dtype=np.float32))
    # value ships as fp8 e3m4, pre-scaled by VSCALE so N(0,1) data sits in the
    # normal range (subnormal flush on HW would otherwise zero |v| < 2^-2);
    # 1/VSCALE is folded into the combined output projection below.
    value = np.ascontiguousarray(
        np.clip(np.asarray(inputs["value"], dtype=np.float32) * VSCALE,
                -15.5, 15.5).astype(ml_dtypes.float8_e3m4))
    consts = {
        k: np.ascontiguousarray(np.asarray(inputs[k], np.float32))
        for k in ["W_off", "b_off", "W_attn", "b_attn", "b_out"]
    }
    W_v = np.asarray(inputs["W_v"], np.float64)
    b_v = np.asarray(inputs["b_v"], np.float64)
    W_out = np.asarray(inputs["W_out"], np.float64).reshape(NH, D, D)
    consts["Wcomb"] = np.ascontiguousarray(
        (np.einsum("ij,hjk->hik", W_v, W_out) / VSCALE)
        .reshape(NH * D, D).astype(ml_dtypes.bfloat16))
    consts["bvW"] = np.ascontiguousarray(
        np.einsum("j,hjk->hk", b_v, W_out).astype(ml_dtypes.bfloat16))
    in_maps = []
    for c in range(NCORES):
        b, s = c // 2, c % 2
        qsl = slice(s * QPC, (s + 1) * QPC)
        in_maps.append({
            "query": np.ascontiguousarray(query[b, qsl]),
            "reference_points": np.ascontiguousarray(refp[b, qsl]),
            "value": np.ascontiguousarray(value[b]),
            **consts,
        })
    return in_maps


def _assemble(outs, shape):
    out = np.zeros(shape, dtype=np.float32)
    for c in range(NCORES):
        b, s = c // 2, c % 2
        out[b, s * QPC:(s + 1) * QPC] = outs[c]["out"]
    return out


def kernel(query, reference_points, value, W_off, b_off, W_attn, b_attn,
           W_v, b_v, W_out, b_out, H=128, W=128, **_unused):
    assert int(H) == HW and int(W) == HW
    from concourse.bass_utils import run_bass_kernel_spmd

    inputs = dict(query=query, reference_points=reference_points, value=value,
                  W_off=W_off, b_off=b_off, W_attn=W_attn, b_attn=b_attn,
                  W_v=W_v, b_v=b_v, W_out=W_out, b_out=b_out)
    in_maps = _make_in_maps(inputs)
    nc = _get_nc()
    res = run_bass_kernel_spmd(nc, in_maps, core_ids=list(range(NCORES)))
    outs = res.results if hasattr(res, "results") else res
    B, Q, _ = np.asarray(query).shape
    return _assemble(outs, (B, Q, D))

